# revision 12
# baseline (speedup 1.0000x reference)
"""GAT-style attention layer (gnn_message_passing) on 8 TRN2 NeuronCores.

Math (reference):
    xf  = X @ W.T                          [N, F1]
    s   = xf @ a0   (att_self,  per-row i)
    t   = xf @ a1   (att_neigh, per-col j)
    att[i,j]   = LeakyReLU_0.2(s_i + t_j)
    E[i,j]     = A[i,j] * exp(att[i,j])      (masked; no max-subtraction
                 needed: |att| < ~25 so exp stays in fp32 range)
    S_j        = sum_i E[i,j]                (softmax axis=0 denominator)
    out[i,g]   = sum_j E[i,j] * xf[j,g] / S_j

Sharding: 1D column (j) shard across 8 cores. Each core owns columns
J_r = [r*1024, (r+1)*1024): it builds E.T[j_local, i] for all i (so the
axis=0 softmax denominator is core-local), aggregates the partial
out[i,:] = sum_{j in J_r} E.T[j,i] * (xf[j,:]/S_j), and one final
ReduceScatter sums partials across cores, handing rank r exactly its
output row block.

The host passes Asc = (A*BIG) as fp16 (exact: A is a 0/1 mask), halving
A's DMA traffic. Per (i-chunk c, j-tile jt) stream unit:
  DMA  : Asc rows (2KB contiguous per partition, fp16)
  DVE  : Am = Asc + (s_i - BIG)   in place (tensor_scalar, 4x mode;
         per-partition s column, so masked entries become ~ -BIG)
  PE   : 8x 128x128 fp16 transposes -> Am.T chunk in PSUM
  DVE  : z = Am.T + t_j           (tensor_scalar from PSUM, 2x mode)
         y = 0.2 * z              (tensor_scalar SBUF, 4x mode)
         w = max(z, y) -> bf16    (tensor_tensor, 2x mode: LeakyReLU)
  ACT  : ET[jt][:, chunk] = Exp(w), accum_out += column sums (fused)
  PE   : aggregation matmuls after all chunks + normalization
All ops avoid scalar_tensor_tensor, which has no fast DVE modes.
"""

import sys

sys.path.insert(0, "/opt/trn_rl_repo")

import numpy as np

import concourse.bass as bass
import concourse.mybir as mybir
from concourse import bacc, tile, masks
from concourse.bass_utils import run_bass_kernel_spmd

N, F, F1 = 8192, 256, 64
NCORES = 8
JL = N // NCORES      # 1024 local columns per core
NT = N // 128         # 64 node tiles (i-tiles)
JT = JL // 128        # 8 local j-tiles per core
FE = F1 + 2           # xf extended with s,t columns
BIG = 30000.0         # additive mask magnitude (fp16-safe)

f32 = mybir.dt.float32
bf16 = mybir.dt.bfloat16
f16 = mybir.dt.float16
Alu = mybir.AluOpType
AF = mybir.ActivationFunctionType


def build_graph(n=N, ncores=NCORES, use_collective=True):
    N_, NCORES_ = n, ncores
    JL_ = N_ // NCORES_
    NT_ = N_ // 128
    JT_ = JL_ // 128
    IPC_ = min(8, NT_)          # i-tiles per chunk
    NCH_ = NT_ // IPC_          # chunks
    CW_ = IPC_ * 128            # chunk width in i
    nc = bacc.Bacc("TRN2", target_bir_lowering=False, num_devices=NCORES_)

    XTl_d = nc.dram_tensor("XTloc", [F, JL_], f32, kind="ExternalInput")
    A_d = nc.dram_tensor("Ash", [N_, JL_], f16, kind="ExternalInput")
    WTe_d = nc.dram_tensor("WTe", [F, FE], f32, kind="ExternalInput")
    out_d = nc.dram_tensor("out", [JL_, F1], f32, kind="ExternalOutput")

    with tile.TileContext(nc) as tc:
        with (
            tc.tile_pool(name="persist", bufs=1) as P,
            tc.tile_pool(name="etp", bufs=1) as ETp,
            tc.tile_pool(name="dram", bufs=1, space="DRAM") as DR,
        ):
            # ---- constants ----
            ident_f16 = P.tile([128, 128], f16)
            masks.make_identity(nc, ident_f16[:])
            ident_f32 = P.tile([128, 128], f32)
            masks.make_identity(nc, ident_f32[:])

            WTe_sb = P.tile([128, 2 * FE], f32)
            nc.sync.dma_start(WTe_sb[:, 0:FE], WTe_d[0:128, :])
            nc.sync.dma_start(WTe_sb[:, FE : 2 * FE], WTe_d[128:256, :])

            # ---- persistent state ----
            ET = [ETp.tile([128, N_], bf16, name=f"et{j}") for j in range(JT_)]
            s_g = P.tile([128, NT_], f32)
            xf_loc = P.tile([128, JT_ * FE], f32)
            xfn = P.tile([128, JT_ * F1], bf16)
            s_cols = P.tile([128, JT_], f32)
            cs_part = P.tile([128, JT_ * NCH_], f32)
            cs = P.tile([128, JT_], f32)
            rinv = P.tile([128, JT_], f32)

            s_loc_dram = DR.tile([JT_, 128], f16)
            s_all_dram = DR.tile(
                [NT_, 128], f16,
                addr_space="Shared" if NCORES_ > 4 else "Local",
            )
            partial_d = DR.tile([N_, F1], f32)
            rs_out_d = DR.tile([JL_, F1], f32)

            # ================= phase 0: local features + s AllGather ========
            with (
                tc.tile_pool(name="xstage", bufs=1) as XS,
                tc.tile_pool(name="xfps", bufs=2, space="PSUM") as XFP,
                tc.tile_pool(name="scps", bufs=1, space="PSUM") as SCP,
            ):
                xtl = XS.tile([128, 2 * JL_], f32, name="xtl")
                nc.sync.dma_start(xtl[:, 0:JL_], XTl_d[0:128, :])
                nc.sync.dma_start(xtl[:, JL_ : 2 * JL_], XTl_d[128:256, :])
                for jt in range(JT_):
                    xfp = XFP.tile([128, FE], f32, name="xfp", bufs=2)
                    nc.tensor.matmul(
                        xfp[:],
                        xtl[:, jt * 128 : (jt + 1) * 128],
                        WTe_sb[:, 0:FE],
                        start=True,
                        stop=False,
                    )
                    nc.tensor.matmul(
                        xfp[:],
                        xtl[:, JL_ + jt * 128 : JL_ + (jt + 1) * 128],
                        WTe_sb[:, FE : 2 * FE],
                        start=False,
                        stop=True,
                    )
                    nc.vector.tensor_copy(
                        xf_loc[:, jt * FE : (jt + 1) * FE], xfp[:]
                    )
                    nc.vector.tensor_copy(
                        s_cols[:, jt : jt + 1],
                        xf_loc[:, jt * FE + F1 : jt * FE + F1 + 1],
                    )

                # local s columns -> rows -> DRAM -> AllGather -> bcast row
                scp = SCP.tile([JT_, 128], f32, name="scp")
                nc.tensor.transpose(scp[:], s_cols[:, 0:JT_], ident_f32[:])
                s_rT = XS.tile([JT_, 128], f16, name="srt", bufs=1)
                nc.vector.tensor_copy(s_rT[:], scp[:])
                nc.sync.dma_start(s_loc_dram[:], s_rT[:])
                nc.gpsimd.collective_compute(
                    "AllGather",
                    Alu.bypass,
                    replica_groups=[list(range(NCORES_))],
                    ins=[s_loc_dram[:].opt()],
                    outs=[s_all_dram[:].opt()],
                )
                # global s back as per-partition columns [128, NT_]
                s_all_sb = XS.tile([NT_, 128], f16, name="sall", bufs=1)
                nc.sync.dma_start(s_all_sb[:], s_all_dram[:])
                sgp = SCP.tile([128, NT_], f16, name="sgp")
                nc.tensor.transpose(
                    sgp[:], s_all_sb[:], ident_f16[0:NT_, 0:NT_]
                )
                nc.vector.tensor_copy(s_g[:], sgp[:])

            # ================= stream: mask+lrelu+exp per (chunk, j-tile) ===
            with (
                tc.tile_pool(name="amsk", bufs=IPC_ + 2) as ABP,
                tc.tile_pool(name="tpps", bufs=4, space="PSUM") as TPP,
                tc.tile_pool(name="zpool", bufs=4) as ZP,
            ):
                for c in range(NCH_):
                    am_tiles = []
                    for q in range(IPC_):
                        tau = c * IPC_ + q
                        am = ABP.tile([128, JL_], f16, name="am")
                        nc.sync.dma_start(
                            am[:], A_d[tau * 128 : (tau + 1) * 128, :]
                        )
                        # Am = Asc + (s_i - BIG), in place (4x single-src)
                        nc.vector.tensor_scalar(
                            am[:], am[:], s_g[:, tau : tau + 1], -BIG,
                            Alu.add, Alu.add,
                        )
                        am_tiles.append(am)
                    for jt in range(JT_):
                        tp = TPP.tile([128, CW_], f16, name="tp")
                        for q in range(IPC_):
                            nc.tensor.transpose(
                                tp[:, q * 128 : (q + 1) * 128],
                                am_tiles[q][:, jt * 128 : (jt + 1) * 128],
                                ident_f16[:],
                            )
                        t_ap = xf_loc[:, jt * FE + F1 + 1 : jt * FE + F1 + 2]
                        z = ZP.tile([128, CW_], f16, name="z")
                        nc.vector.tensor_scalar(
                            z[:], tp[:], t_ap, None, Alu.add
                        )
                        y = ZP.tile([128, CW_], f16, name="y")
                        nc.vector.tensor_scalar(
                            y[:], z[:], 0.2, None, Alu.mult
                        )
                        w = ZP.tile([128, CW_], bf16, name="w")
                        nc.vector.tensor_tensor(
                            w[:], z[:], y[:], Alu.max
                        )
                        nc.scalar.activation(
                            ET[jt][:, c * CW_ : (c + 1) * CW_],
                            w[:],
                            AF.Exp,
                            accum_out=cs_part[:, jt * NCH_ + c : jt * NCH_ + c + 1],
                        )

            # ================= tail: normalize, aggregate, reduce ============
            with (
                tc.tile_pool(name="aggps", bufs=6, space="PSUM") as AGP,
                tc.tile_pool(name="ocp", bufs=1) as OCP,
            ):
                for jt in range(JT_):
                    nc.vector.tensor_reduce(
                        cs[:, jt : jt + 1],
                        cs_part[:, jt * NCH_ : (jt + 1) * NCH_],
                        axis=mybir.AxisListType.X,
                        op=Alu.add,
                    )
                nc.vector.reciprocal(rinv[:], cs[:])
                for jt in range(JT_):
                    nc.vector.tensor_scalar(
                        xfn[:, jt * F1 : (jt + 1) * F1],
                        xf_loc[:, jt * FE : jt * FE + F1],
                        rinv[:, jt : jt + 1],
                        None,
                        Alu.mult,
                    )
                stage = OCP.tile([128, NT_ * F1], f32, name="stage")
                for b in range(NT_):
                    ag = AGP.tile([128, F1], f32, name="ag")
                    for jt in range(JT_):
                        nc.tensor.matmul(
                            ag[:],
                            ET[jt][:, b * 128 : (b + 1) * 128],
                            xfn[:, jt * F1 : (jt + 1) * F1],
                            start=(jt == 0),
                            stop=(jt == JT_ - 1),
                        )
                    nc.scalar.copy(stage[:, b * F1 : (b + 1) * F1], ag[:])
                # one batched DMA: stage[p, b*F1+g] -> partial_d[b*128+p, g]
                nc.sync.dma_start(
                    partial_d[:].rearrange("(b p) g -> p b g", p=128),
                    stage[:].rearrange("p (b g) -> p b g", g=F1),
                )

                if use_collective:
                    nc.gpsimd.collective_compute(
                        "ReduceScatter",
                        Alu.add,
                        replica_groups=[list(range(NCORES_))],
                        ins=[partial_d[:].opt()],
                        outs=[rs_out_d[:].opt()],
                    )
                    nc.sync.dma_start(out_d[:], rs_out_d[:])
                else:
                    # timing-model variant (TimelineSim is single-core only)
                    nc.sync.dma_start(out_d[:], partial_d[0:JL_, :])

    nc.compile()
    return nc


_GRAPH = None


def make_in_maps(X, A, W, a):
    X = np.asarray(X, dtype=np.float32)
    A = np.asarray(A, dtype=np.float32)
    W = np.asarray(W, dtype=np.float32)
    a = np.asarray(a, dtype=np.float32)

    WT = W.T.astype(np.float32)                      # [256, 64]
    WTe = np.concatenate([WT, WT @ a[0], WT @ a[1]], axis=1)  # [256, 66]
    WTe = np.ascontiguousarray(WTe, dtype=np.float32)

    in_maps = []
    for r in range(NCORES):
        in_maps.append(
            {
                "XTloc": np.ascontiguousarray(X[r * JL : (r + 1) * JL].T),
                "Ash": np.ascontiguousarray(
                    (A[:, r * JL : (r + 1) * JL] * BIG).astype(np.float16)
                ),
                "WTe": WTe,
            }
        )
    return in_maps


def kernel(X, A, W, a):
    global _GRAPH
    if _GRAPH is None:
        _GRAPH = build_graph()
    nc = _GRAPH

    in_maps = make_in_maps(X, A, W, a)
    res = run_bass_kernel_spmd(nc, in_maps, list(range(NCORES)))
    out = np.concatenate(
        [res.results[r]["out"] for r in range(NCORES)], axis=0
    )
    return out.astype(np.float32)
